# revision 34
# baseline (speedup 1.0000x reference)
"""CMPLoss kernel for Trainium2 (8 NeuronCores, SPMD row-sharded).

Reference semantics (B = 8192, probs [B,B] f32, labels [B] int):
    p_true[i] = probs[i, labels[i]]
    sel[i,j]  = (labels[j] != labels[i]) & (probs[i,j] > p_true[i])
    denom[i]  = sum_j sel ? probs[i,j] : 0
    contrib[i]= any(sel[i,:]) ? p_true[i] / (denom[i] + 1e-10) : 0
    out       = sum(contrib) / B

The kernel is HBM-bandwidth bound (each core must stream its whole row
shard once), so the host quantizes probs to uint16 fixed-point
(q = rint(x * 65535)) before upload: half the f32 bytes, and the DVE's
2x_1P packed mode doubles elementwise throughput for 16-bit dtypes.
Device computes  A[i] = sum_j q[i,j] * [q[i,j] > qp[i]]  with one fused
DVE scalar_tensor_tensor per chunk ((x is_gt p) mult x with accum_out =
per-partition row sum; the p scalar is the f32-held integer qp[i], so
the compare is exact in the u16 grid).

Host-side corrections (both tiny, O(B) and O(T*B)):
  * same-label columns:  C[i] = sum_{j: labels[j]==labels[i]} q*[q > qp[i]]
    (~B pairs in expectation), so denom_q = (A - C) / 65535.
  * quantization tail:  rows whose contrib is dominated by a few
    elements near the row max (denom_q < T = 64, ~60 rows) are scrambled
    by ANY quantization, so they are recomputed exactly from the f32
    input on host.  Residual rel err vs f32 reference: 8.4e-5 (measured,
    seed-0 inputs; T=0 would already give 6.2e-4).

has_any[i] for the remaining rows is implied by denom_q >= T (a row with
no selected element has |A - C| < 1 on the u16-integer scale).

Sharding: probs row-sharded 1024 rows/core across 8 cores; qp slice
replicated per-core (tiny); per-row partial sums returned; host finalizes.
"""

import numpy as np

import concourse.bacc as bacc
import concourse.mybir as mybir
import concourse.tile as tile
from concourse.bass_utils import run_bass_kernel_spmd

B = 8192
N_CORES = 8
P = 128  # SBUF partitions
ROWS_PER_CORE = B // N_CORES  # 1024

# Quantized-probs dtype: "u16" (fixed-point rint(x*65535)) or "f16" (IEEE
# half).  u16 is finer near 1.0 but the DVE has no packed-mode uop for
# integer dtypes (STT runs 1x); f16 gets the 2x_1P packed mode.
QMODE = "f16"
# Device compute variant:
#   "stt":   one fused scalar_tensor_tensor per chunk (runs 1x on DVE; the
#            DVE alone is then the bottleneck at ~8.75us/128-row block).
#   "ts2":   two tensor_scalar add-reduce ops per chunk (lowered to
#            TENSOR_SCALAR_CACHE_REDUCE, which also runs 1x: worse).
#   "split": column-split every chunk between the DVE (fused STT on the
#            first DVE_COLS columns) and the scalar/ACT engine (Relu and
#            Sign activation-accumulate passes on the rest).  Both engines
#            run at 1 elem/lane/cycle, but in parallel the per-block wall
#            time drops to ~max(DVE_COLS/0.96GHz, 2*ACT_COLS/1.2GHz),
#            right at the DMA streaming rate.
#   "split3": like "split" but the count moves from a second ACT pass
#            (Sign) to the Pool/GPSIMD engine as a plain tensor_scalar
#            is_gt/add-reduce, so each of the three engines runs ONE
#            1x pass per chunk over its column share.
VARIANT = "split"
# Column shares per chunk (fractions of the chunk width, 64-aligned).
# Measured rates: DVE fused STT ~1.04 ns/col + 750 ns/chunk; ACT
# activation-accumulate ~1.08 ns/col + ~900 ns/chunk; Pool unknown.
DVE_FRAC = 5376 / 8192.0  # used by "split" (DVE vs ACT two-pass)
# split3: DVE gets S3_DVE of the chunk (fused STT).  ACT computes
# S = sum relu(x-p) over ALL remaining cols, plus the count (Sign) for
# the first S3_ACTCNT share; Pool counts the rest with is_gt/add.
S3_DVE = 4480 / 8192.0
S3_ACTCNT = 192 / 8192.0
QSCALE = np.float32(65535.0)
SUSPECT_T = 64.0  # rows with denom below this are recomputed exactly on host

_NC_CACHE = {}


NSPLIT = 2  # the last block is split column-wise into NSPLIT chunks


def chunk_plan(nblocks, ncols):
    """(block, col0, col1) chunks.  Full-width ops minimize both DVE per-op
    overhead and the ~0.6us serial per-DMA setup on the (FIFO) HWDGE ring;
    only the last block is split, halving the compute tail that trails the
    DMA stream.  The host repacks the split block chunk-contiguously in DRAM
    (see _pack_shard), so every DMA reads a fully contiguous range."""
    if nblocks < 1 or ncols % NSPLIT != 0:
        return [(b, 0, ncols) for b in range(nblocks)]
    q = ncols // NSPLIT
    # Split the first block (compute starts as soon as the first half-chunk
    # lands, ~2.7us earlier) and the last block (halves the compute tail
    # that trails the DMA stream).
    split = {0, nblocks - 1}
    chunks = []
    for b in range(nblocks):
        if b in split:
            chunks += [(b, c * q, (c + 1) * q) for c in range(NSPLIT)]
        else:
            chunks.append((b, 0, ncols))
    return chunks


def _pack_shard(shard, nblocks, ncols):
    """Repack split blocks chunk-contiguously: block b's chunk c occupies the
    flat range [(b*P*ncols + c0*P), ...) as a row-major [P, c1-c0] array."""
    q = ncols // NSPLIT
    split = {0, nblocks - 1}
    parts = []
    for b in range(nblocks):
        blk = shard[b * P : (b + 1) * P]
        if b in split and ncols % NSPLIT == 0 and nblocks >= 1:
            parts.append(
                np.ascontiguousarray(
                    blk.reshape(P, NSPLIT, q).transpose(1, 0, 2)
                ).reshape(-1)
            )
        else:
            parts.append(blk.reshape(-1))
    return np.concatenate(parts)


def dve_cols(width):
    """DVE's column share of a chunk of `width` cols (64-aligned)."""
    frac = S3_DVE if VARIANT == "split3" else DVE_FRAC
    return int(round(width * frac / 64.0)) * 64


def act_cnt_cols(width):
    """ACT's count (Sign) column share of a chunk (split3; 64-aligned)."""
    return int(round(width * S3_ACTCNT / 64.0)) * 64


def build_bass(rows_per_core=ROWS_PER_CORE, ncols=B):
    """SPMD program (identical on all cores): stream row-blocks of the f16
    probs from DRAM; for each chunk the DVE computes the fused masked sum
    A_dve = sum_j x*[x > p] over its column share, and the ACT engine
    computes S = sum relu(x - p) and G = sum sign(x - p) over the rest.

    probs is passed pre-packed by _pack_shard (chunk-contiguous), so every
    DMA below reads a contiguous DRAM range."""
    nblocks = rows_per_core // P
    chunks = chunk_plan(nblocks, ncols)
    f32 = mybir.dt.float32
    u16 = mybir.dt.float16 if QMODE == "f16" else mybir.dt.uint16
    nc = bacc.Bacc()
    probs_in = nc.declare_dram_parameter(
        "probs", [rows_per_core * ncols], u16, isOutput=False
    )
    n_dve = len(chunks)
    split = VARIANT in ("split", "split3")
    # pt_all[:, 0:nb] = p (DVE scalar operand); pt_all[:, nb:2nb] = -p
    # (ACT bias).
    ptw = 2 * nblocks if split else nblocks
    pt_in = nc.declare_dram_parameter("p_true_t", [P, ptw], f32, isOutput=False)
    if VARIANT == "split3":
        nacc = 4 * n_dve
    elif split:
        nacc = 3 * n_dve
    else:
        nacc = n_dve
    a_out = nc.declare_dram_parameter("a_out", [P, nacc], f32, isOutput=True)
    if VARIANT == "ts2":
        n_out = nc.declare_dram_parameter("n_out", [P, n_dve], f32, isOutput=True)

    relu = mybir.ActivationFunctionType.Relu
    sign = mybir.ActivationFunctionType.Sign
    copyf = mybir.ActivationFunctionType.Copy

    with tile.TileContext(nc) as tc:
        with (
            tc.tile_pool(name="xp", bufs=4) as xp,
            tc.tile_pool(name="mp", bufs=1) as mp,
        ):
            pt = mp.tile([P, ptw], f32)
            # SWDGE path: keeps the tiny p_true load off the HWDGE ring that
            # streams the probs blocks.
            nc.gpsimd.dma_start(pt[:], pt_in[:])
            acc = mp.tile([P, nacc], f32)
            if VARIANT == "ts2":
                accn = mp.tile([P, n_dve], f32)
            scr = mp.tile([P, ncols], u16)
            dummy = mp.tile([P, 1], f32)
            # Wait-absorbers: the fused STT op has too few HW sync-wait slots
            # for Tile's semaphores, and letting bacc legalize multi-waits
            # into event-sem chains adds ~2.5us of DMA->DVE completion-signal
            # latency per block (measured).  A tiny DVE read of each tile
            # carries the wait instead; the engine's vector clock then covers
            # the STT's deps for free.
            nc.vector.tensor_copy(dummy[:], pt[:, 0:1])
            if split:
                dummy_s = mp.tile([P, 1], f32)
                nc.scalar.activation(dummy_s[:], pt[:, 0:1], copyf)
            if VARIANT == "split3":
                scr_g = mp.tile([P, ncols], u16)
                dummy_g = mp.tile([P, 1], f32)
                nc.gpsimd.tensor_copy(dummy_g[:], pt[:, 0:1])
            cur_block = None
            x = None
            for ci, (b, c0, c1) in enumerate(chunks):
                if b != cur_block:
                    x = xp.tile([P, ncols], u16, tag="x")
                    cur_block = b
                src = probs_in[
                    b * P * ncols + c0 * P : b * P * ncols + c1 * P
                ].rearrange("(p m) -> p m", p=P)
                # Alternate chunk loads between the HWDGE (sync) and SWDGE
                # (gpsimd) rings: the per-op descriptor-gen/doorbell bubble
                # of one ring overlaps the other's data, keeping the 16
                # shared SDMA engines closer to line rate.
                if b % 2 and VARIANT in ("split", "split3"):
                    nc.gpsimd.dma_start(x[:, c0:c1], src)
                else:
                    nc.sync.dma_start(x[:, c0:c1], src)
                nc.vector.tensor_copy(dummy[:], x[:, c0 : c0 + 1])
                if VARIANT == "split3":
                    dw = dve_cols(c1 - c0)
                    uw = act_cnt_cols(c1 - c0)
                    m = c0 + dw
                    m2 = m + uw
                    nc.vector.scalar_tensor_tensor(
                        out=scr[:, c0:m],
                        in0=x[:, c0:m],
                        scalar=pt[:, b : b + 1],
                        in1=x[:, c0:m],
                        op0=mybir.AluOpType.is_gt,
                        op1=mybir.AluOpType.mult,
                        accum_out=acc[:, ci : ci + 1],
                    )
                    # ACT: S = sum relu(x - p) over ALL non-DVE cols [m, c1)
                    nc.scalar.activation(dummy_s[:], x[:, m : m + 1], copyf)
                    nc.scalar.activation(
                        scr[:, m:c1],
                        x[:, m:c1],
                        relu,
                        bias=pt[:, nblocks + b : nblocks + b + 1],
                        accum_out=acc[:, n_dve + ci : n_dve + ci + 1],
                    )
                    # ACT: G = sum sign(x - p) over [m, m2): count for the
                    # first uw non-DVE cols: cnt = (G + uw - ties)/2
                    if uw:
                        nc.scalar.activation(
                            scr[:, m:m2],
                            x[:, m:m2],
                            sign,
                            bias=pt[:, nblocks + b : nblocks + b + 1],
                            accum_out=acc[:, 2 * n_dve + ci : 2 * n_dve + ci + 1],
                        )
                    # Pool: cnt = sum [x > p] over the remaining [m2, c1)
                    nc.gpsimd.tensor_copy(dummy_g[:], x[:, m2 : m2 + 1])
                    nc.gpsimd.tensor_scalar(
                        out=scr_g[:, m2:c1],
                        in0=x[:, m2:c1],
                        scalar1=pt[:, b : b + 1],
                        scalar2=0.0,
                        op0=mybir.AluOpType.is_gt,
                        op1=mybir.AluOpType.add,
                        accum_out=acc[:, 3 * n_dve + ci : 3 * n_dve + ci + 1],
                    )
                elif split:
                    dw = dve_cols(c1 - c0)
                    m = c0 + dw
                    nc.vector.scalar_tensor_tensor(
                        out=scr[:, c0:m],
                        in0=x[:, c0:m],
                        scalar=pt[:, b : b + 1],
                        in1=x[:, c0:m],
                        op0=mybir.AluOpType.is_gt,
                        op1=mybir.AluOpType.mult,
                        accum_out=acc[:, ci : ci + 1],
                    )
                    nc.scalar.activation(
                        scr[:, m:c1],
                        x[:, m:c1],
                        relu,
                        bias=pt[:, nblocks + b : nblocks + b + 1],
                        accum_out=acc[:, n_dve + ci : n_dve + ci + 1],
                    )
                    nc.scalar.activation(
                        scr[:, m:c1],
                        x[:, m:c1],
                        sign,
                        bias=pt[:, nblocks + b : nblocks + b + 1],
                        accum_out=acc[:, 2 * n_dve + ci : 2 * n_dve + ci + 1],
                    )
                elif VARIANT == "ts2":
                    # For plain tensor_scalar with accum_out, op1 IS the
                    # reduce op: accum = reduce_op1(in0 op0 scalar1), then
                    # op1 scalar2.  Two add-reduce passes:
                    #   M = sum max(x, p);  N = sum [x > p]
                    # Host recovers A = sum x*[x>p] = M + p*(N - ncols).
                    nc.vector.tensor_scalar(
                        out=scr[:, c0:c1],
                        in0=x[:, c0:c1],
                        scalar1=pt[:, b : b + 1],
                        scalar2=0.0,
                        op0=mybir.AluOpType.max,
                        op1=mybir.AluOpType.add,
                        accum_out=acc[:, ci : ci + 1],
                    )
                    nc.vector.tensor_scalar(
                        out=scr[:, c0:c1],
                        in0=x[:, c0:c1],
                        scalar1=pt[:, b : b + 1],
                        scalar2=0.0,
                        op0=mybir.AluOpType.is_gt,
                        op1=mybir.AluOpType.add,
                        accum_out=accn[:, ci : ci + 1],
                    )
                else:
                    nc.vector.scalar_tensor_tensor(
                        out=scr[:, c0:c1],
                        in0=x[:, c0:c1],
                        scalar=pt[:, b : b + 1],
                        in1=x[:, c0:c1],
                        op0=mybir.AluOpType.is_gt,
                        op1=mybir.AluOpType.mult,
                        accum_out=acc[:, ci : ci + 1],
                    )
            nc.sync.dma_start(a_out[:], acc[:])
            if VARIANT == "ts2":
                nc.sync.dma_start(n_out[:], accn[:])
    # Legalize for TRN2 (at most 1 sem wait per instruction -> event sems).
    nc.compile()
    return nc


def _get_nc():
    key = (ROWS_PER_CORE, B)
    if key not in _NC_CACHE:
        _NC_CACHE[key] = build_bass()
    return _NC_CACHE[key]


def _act_col_ranges():
    """Per block: the (absolute-column) ranges counted via ACT Sign (whose
    exact ties need a host-side correction)."""
    nblocks = ROWS_PER_CORE // P
    chunks = chunk_plan(nblocks, B)
    out = {b: [] for b in range(nblocks)}
    for b, c0, c1 in chunks:
        m = c0 + dve_cols(c1 - c0)
        m2 = m + act_cnt_cols(c1 - c0) if VARIANT == "split3" else c1
        out[b].append((m, m2))
    return out


def _device_A(probs_q, p_q, eq_cnt=None, **run_kwargs):
    """Run the SPMD kernel on 8 cores; return A [B] float64 and the raw
    BassKernelResults (for profiling).  eq_cnt [B]: per-row count of exact
    ties q(x) == p within the ACT column ranges (split variant only)."""
    nblocks = ROWS_PER_CORE // P
    split = VARIANT in ("split", "split3")
    in_maps = []
    for k in range(N_CORES):
        r0 = k * ROWS_PER_CORE
        shard = _pack_shard(probs_q[r0 : r0 + ROWS_PER_CORE], nblocks, B)
        # p_true laid out [partition, block]: ptt[q, b] = p_q[r0 + b*P + q]
        ptt = np.ascontiguousarray(
            p_q[r0 : r0 + ROWS_PER_CORE].reshape(nblocks, P).T
        )
        if split:
            ptt = np.ascontiguousarray(np.concatenate([ptt, -ptt], axis=1))
        in_maps.append({"probs": shard, "p_true_t": ptt})
    res = run_bass_kernel_spmd(
        _get_nc(), in_maps, core_ids=list(range(N_CORES)), **run_kwargs
    )
    chunks = chunk_plan(nblocks, B)
    n_dve = len(chunks)
    A = np.empty(B, np.float64)
    for k in range(N_CORES):
        a = res.results[k]["a_out"]  # [P, nacc]
        if VARIANT == "ts2":
            a = np.concatenate([a, res.results[k]["n_out"]], axis=1)
        a_shard = np.zeros((nblocks, P), np.float64)
        n_act = np.zeros((nblocks, P), np.float64)
        pool_cnt = np.zeros((nblocks, P), np.float64)
        for ci, (b, c0, c1) in enumerate(chunks):
            a_shard[b] += a[:, ci].astype(np.float64)
            if VARIANT == "split3":
                a_shard[b] += a[:, n_dve + ci].astype(np.float64)  # S
                n_act[b] += a[:, 2 * n_dve + ci].astype(np.float64) + (
                    act_cnt_cols(c1 - c0)
                )
                pool_cnt[b] += a[:, 3 * n_dve + ci].astype(np.float64)
            elif split:
                # + S (relu accum); G (sign accum) + n_cols feeds the
                # count term below
                a_shard[b] += a[:, n_dve + ci].astype(np.float64)
                n_act[b] += a[:, 2 * n_dve + ci].astype(np.float64) + (
                    (c1 - c0) - dve_cols(c1 - c0)
                )
            elif VARIANT == "ts2":
                n_act[b] += a[:, n_dve + ci].astype(np.float64) - (c1 - c0)
        p_shard = (
            p_q[k * ROWS_PER_CORE : (k + 1) * ROWS_PER_CORE]
            .astype(np.float64)
            .reshape(nblocks, P)
        )
        if split:
            # count(x > p) = (G + n_sign_cols - ties) / 2  (+ Pool's count)
            eq = (
                eq_cnt[k * ROWS_PER_CORE : (k + 1) * ROWS_PER_CORE].reshape(
                    nblocks, P
                )
                if eq_cnt is not None
                else 0.0
            )
            a_shard += p_shard * ((n_act - eq) / 2.0 + pool_cnt)
        elif VARIANT == "ts2":
            a_shard += p_shard * n_act
        A[k * ROWS_PER_CORE : (k + 1) * ROWS_PER_CORE] = a_shard.reshape(-1)
    return A, res


def _same_label_correction(probs_u16, labels, p_q):
    """C[i] = sum over j with labels[j]==labels[i] of q*[q > qp[i]], exactly
    on the u16 integer scale (uint16 -> float64 is exact)."""
    C = np.zeros(B, np.float64)
    order = np.argsort(labels, kind="stable")
    ls = labels[order]
    bounds = np.flatnonzero(np.r_[True, ls[1:] != ls[:-1], True])
    for s, e in zip(bounds[:-1], bounds[1:]):
        g = order[s:e]
        sub = probs_u16[np.ix_(g, g)].astype(np.float64)
        pt = p_q[g].astype(np.float64)[:, None]
        C[g] = np.sum(np.where(sub > pt, sub, 0.0), axis=1)
    return C


def _exact_rows(probs, labels, p_true, rows):
    """Exact f32-input contrib for the given rows (float64 math)."""
    sub = probs[rows].astype(np.float64)
    pt = p_true[rows].astype(np.float64)[:, None]
    sel = (labels[None, :] != labels[rows][:, None]) & (sub > pt)
    den = np.where(sel, sub, 0.0).sum(axis=1)
    has = sel.any(axis=1)
    return np.where(has, p_true[rows].astype(np.float64) / (den + 1e-10), 0.0)


def run(probs, labels, **run_kwargs):
    """Full computation; returns (scalar ndarray float32, BassKernelResults)."""
    probs = np.ascontiguousarray(np.asarray(probs, dtype=np.float32))
    labels = np.asarray(labels).astype(np.int64)
    assert probs.shape == (B, B) and labels.shape == (B,)

    p_true = probs[np.arange(B), labels]  # f32 [B]
    # Quantize once; the SAME array feeds the device, the same-label
    # correction, and the suspect-row detection, so they agree exactly.
    if QMODE == "f16":
        probs_q = probs.astype(np.float16)
        p_q = p_true  # f32 scalar operand, compare is exact
        scale = np.float64(1.0)
    else:
        probs_q = np.rint(probs * QSCALE).astype(np.uint16)
        p_q = np.rint(p_true * QSCALE)  # integer-valued f32 scalar operand
        scale = np.float64(QSCALE)

    eq_cnt = None
    if VARIANT == "split":
        # sign(x - p) is 0 on exact ties, which only happen when p is
        # itself representable in the quantized dtype (~1 row in 8k);
        # count those ties exactly for the count reconstruction.
        eq_cnt = np.zeros(B, np.float64)
        rep = p_q == p_q.astype(probs_q.dtype).astype(p_q.dtype)
        if rep.any():
            ranges = _act_col_ranges()
            for i in np.flatnonzero(rep):
                b = (i % ROWS_PER_CORE) // P
                row = probs_q[i].astype(np.float64)
                eq_cnt[i] = sum(
                    float(np.sum(row[a0:a1] == np.float64(p_q[i])))
                    for a0, a1 in ranges[b]
                )

    A, res = _device_A(probs_q, p_q, eq_cnt, **run_kwargs)
    C = _same_label_correction(probs_q, labels, p_q)

    denom = (A - C) / scale
    contrib = np.where(
        denom > 0.25, p_true.astype(np.float64) / (denom + 1e-10), 0.0
    )
    suspect = denom < SUSPECT_T
    if suspect.any():
        rows = np.flatnonzero(suspect)
        contrib[rows] = _exact_rows(probs, labels, p_true, rows)
    out = np.float32(contrib.sum() / B)
    return np.array(out, dtype=np.float32), res


def kernel(probs, labels):
    out, _ = run(probs, labels)
    return out


# revision 35
# speedup vs baseline: 1.2131x; 1.2131x over previous
"""CMPLoss kernel for Trainium2 (8 NeuronCores, SPMD row-sharded).

Reference semantics (B = 8192, probs [B,B] f32, labels [B] int):
    p_true[i] = probs[i, labels[i]]
    sel[i,j]  = (labels[j] != labels[i]) & (probs[i,j] > p_true[i])
    denom[i]  = sum_j sel ? probs[i,j] : 0
    contrib[i]= any(sel[i,:]) ? p_true[i] / (denom[i] + 1e-10) : 0
    out       = sum(contrib) / B

The kernel is HBM-bandwidth bound (each core must stream its whole row
shard once), so the host quantizes probs to uint16 fixed-point
(q = rint(x * 65535)) before upload: half the f32 bytes, and the DVE's
2x_1P packed mode doubles elementwise throughput for 16-bit dtypes.
Device computes  A[i] = sum_j q[i,j] * [q[i,j] > qp[i]]  with one fused
DVE scalar_tensor_tensor per chunk ((x is_gt p) mult x with accum_out =
per-partition row sum; the p scalar is the f32-held integer qp[i], so
the compare is exact in the u16 grid).

Host-side corrections (both tiny, O(B) and O(T*B)):
  * same-label columns:  C[i] = sum_{j: labels[j]==labels[i]} q*[q > qp[i]]
    (~B pairs in expectation), so denom_q = (A - C) / 65535.
  * quantization tail:  rows whose contrib is dominated by a few
    elements near the row max (denom_q < T = 64, ~60 rows) are scrambled
    by ANY quantization, so they are recomputed exactly from the f32
    input on host.  Residual rel err vs f32 reference: 8.4e-5 (measured,
    seed-0 inputs; T=0 would already give 6.2e-4).

has_any[i] for the remaining rows is implied by denom_q >= T (a row with
no selected element has |A - C| < 1 on the u16-integer scale).

Sharding: probs row-sharded 1024 rows/core across 8 cores; qp slice
replicated per-core (tiny); per-row partial sums returned; host finalizes.
"""

import numpy as np

import concourse.bacc as bacc
import concourse.mybir as mybir
import concourse.tile as tile
from concourse.bass_utils import run_bass_kernel_spmd

B = 8192
N_CORES = 8
P = 128  # SBUF partitions
ROWS_PER_CORE = B // N_CORES  # 1024

# Quantized-probs dtype: "u16" (fixed-point rint(x*65535)) or "f16" (IEEE
# half).  u16 is finer near 1.0 but the DVE has no packed-mode uop for
# integer dtypes (STT runs 1x); f16 gets the 2x_1P packed mode.
QMODE = "f16"
# Device compute variant:
#   "stt":   one fused scalar_tensor_tensor per chunk (runs 1x on DVE; the
#            DVE alone is then the bottleneck at ~8.75us/128-row block).
#   "ts2":   two tensor_scalar add-reduce ops per chunk (lowered to
#            TENSOR_SCALAR_CACHE_REDUCE, which also runs 1x: worse).
#   "split": column-split every chunk between the DVE (fused STT on the
#            first DVE_COLS columns) and the scalar/ACT engine (Relu and
#            Sign activation-accumulate passes on the rest).  Both engines
#            run at 1 elem/lane/cycle, but in parallel the per-block wall
#            time drops to ~max(DVE_COLS/0.96GHz, 2*ACT_COLS/1.2GHz),
#            right at the DMA streaming rate.
#   "split3": like "split" but the count moves from a second ACT pass
#            (Sign) to the Pool/GPSIMD engine as a plain tensor_scalar
#            is_gt/add-reduce, so each of the three engines runs ONE
#            1x pass per chunk over its column share.
VARIANT = "split"
# Column shares per chunk (fractions of the chunk width, 64-aligned).
# Measured rates: DVE fused STT ~1.04 ns/col + 750 ns/chunk; ACT
# activation-accumulate ~1.08 ns/col + ~900 ns/chunk; Pool unknown.
DVE_FRAC = 5440 / 8192.0  # used by "split" (DVE vs ACT two-pass)
# split3: DVE gets S3_DVE of the chunk (fused STT).  ACT computes
# S = sum relu(x-p) over ALL remaining cols, plus the count (Sign) for
# the first S3_ACTCNT share; Pool counts the rest with is_gt/add.
S3_DVE = 4480 / 8192.0
S3_ACTCNT = 192 / 8192.0
QSCALE = np.float32(65535.0)
SUSPECT_T = 64.0  # rows with denom below this are recomputed exactly on host

_NC_CACHE = {}


NSPLIT = 2  # the last block is split column-wise into NSPLIT chunks


def chunk_plan(nblocks, ncols):
    """(block, col0, col1) chunks.  Full-width ops minimize both DVE per-op
    overhead and the ~0.6us serial per-DMA setup on the (FIFO) HWDGE ring;
    only the last block is split, halving the compute tail that trails the
    DMA stream.  The host repacks the split block chunk-contiguously in DRAM
    (see _pack_shard), so every DMA reads a fully contiguous range."""
    if nblocks < 1 or ncols % NSPLIT != 0:
        return [(b, 0, ncols) for b in range(nblocks)]
    q = ncols // NSPLIT
    # Split the first block (compute starts as soon as the first half-chunk
    # lands, ~2.7us earlier) and the last block (halves the compute tail
    # that trails the DMA stream).
    split = {0, nblocks - 1}
    chunks = []
    for b in range(nblocks):
        if b in split:
            chunks += [(b, c * q, (c + 1) * q) for c in range(NSPLIT)]
        else:
            chunks.append((b, 0, ncols))
    return chunks


def _pack_shard(shard, nblocks, ncols):
    """Repack split blocks chunk-contiguously: block b's chunk c occupies the
    flat range [(b*P*ncols + c0*P), ...) as a row-major [P, c1-c0] array."""
    q = ncols // NSPLIT
    split = {0, nblocks - 1}
    parts = []
    for b in range(nblocks):
        blk = shard[b * P : (b + 1) * P]
        if b in split and ncols % NSPLIT == 0 and nblocks >= 1:
            parts.append(
                np.ascontiguousarray(
                    blk.reshape(P, NSPLIT, q).transpose(1, 0, 2)
                ).reshape(-1)
            )
        else:
            parts.append(blk.reshape(-1))
    return np.concatenate(parts)


def dve_cols(width):
    """DVE's column share of a chunk of `width` cols (64-aligned)."""
    frac = S3_DVE if VARIANT == "split3" else DVE_FRAC
    return int(round(width * frac / 64.0)) * 64


def act_cnt_cols(width):
    """ACT's count (Sign) column share of a chunk (split3; 64-aligned)."""
    return int(round(width * S3_ACTCNT / 64.0)) * 64


def build_bass(rows_per_core=ROWS_PER_CORE, ncols=B):
    """SPMD program (identical on all cores): stream row-blocks of the f16
    probs from DRAM; for each chunk the DVE computes the fused masked sum
    A_dve = sum_j x*[x > p] over its column share, and the ACT engine
    computes S = sum relu(x - p) and G = sum sign(x - p) over the rest.

    probs is passed pre-packed by _pack_shard (chunk-contiguous), so every
    DMA below reads a contiguous DRAM range."""
    nblocks = rows_per_core // P
    chunks = chunk_plan(nblocks, ncols)
    f32 = mybir.dt.float32
    u16 = mybir.dt.float16 if QMODE == "f16" else mybir.dt.uint16
    nc = bacc.Bacc()
    probs_in = nc.declare_dram_parameter(
        "probs", [rows_per_core * ncols], u16, isOutput=False
    )
    n_dve = len(chunks)
    split = VARIANT in ("split", "split3")
    # pt_all[:, 0:nb] = p (DVE scalar operand); pt_all[:, nb:2nb] = -p
    # (ACT bias).
    ptw = 2 * nblocks if split else nblocks
    pt_in = nc.declare_dram_parameter("p_true_t", [P, ptw], f32, isOutput=False)
    if VARIANT == "split3":
        nacc = 4 * n_dve
    elif split:
        nacc = 3 * n_dve
    else:
        nacc = n_dve
    a_out = nc.declare_dram_parameter("a_out", [P, nacc], f32, isOutput=True)
    if VARIANT == "ts2":
        n_out = nc.declare_dram_parameter("n_out", [P, n_dve], f32, isOutput=True)

    relu = mybir.ActivationFunctionType.Relu
    sign = mybir.ActivationFunctionType.Sign
    copyf = mybir.ActivationFunctionType.Copy

    with tile.TileContext(nc) as tc:
        with (
            tc.tile_pool(name="xp", bufs=4) as xp,
            tc.tile_pool(name="mp", bufs=1) as mp,
        ):
            pt = mp.tile([P, ptw], f32)
            # SWDGE path: keeps the tiny p_true load off the HWDGE ring that
            # streams the probs blocks.
            nc.gpsimd.dma_start(pt[:], pt_in[:])
            acc = mp.tile([P, nacc], f32)
            if VARIANT == "ts2":
                accn = mp.tile([P, n_dve], f32)
            scr = mp.tile([P, ncols], u16)
            dummy = mp.tile([P, 1], f32)
            # Wait-absorbers: the fused STT op has too few HW sync-wait slots
            # for Tile's semaphores, and letting bacc legalize multi-waits
            # into event-sem chains adds ~2.5us of DMA->DVE completion-signal
            # latency per block (measured).  A tiny DVE read of each tile
            # carries the wait instead; the engine's vector clock then covers
            # the STT's deps for free.
            nc.vector.tensor_copy(dummy[:], pt[:, 0:1])
            if split:
                dummy_s = mp.tile([P, 1], f32)
                nc.scalar.activation(dummy_s[:], pt[:, 0:1], copyf)
            if VARIANT == "split3":
                scr_g = mp.tile([P, ncols], u16)
                dummy_g = mp.tile([P, 1], f32)
                nc.gpsimd.tensor_copy(dummy_g[:], pt[:, 0:1])
            cur_block = None
            x = None
            for ci, (b, c0, c1) in enumerate(chunks):
                if b != cur_block:
                    x = xp.tile([P, ncols], u16, tag="x")
                    cur_block = b
                src = probs_in[
                    b * P * ncols + c0 * P : b * P * ncols + c1 * P
                ].rearrange("(p m) -> p m", p=P)
                nc.sync.dma_start(x[:, c0:c1], src)
                nc.vector.tensor_copy(dummy[:], x[:, c0 : c0 + 1])
                if VARIANT == "split3":
                    dw = dve_cols(c1 - c0)
                    uw = act_cnt_cols(c1 - c0)
                    m = c0 + dw
                    m2 = m + uw
                    nc.vector.scalar_tensor_tensor(
                        out=scr[:, c0:m],
                        in0=x[:, c0:m],
                        scalar=pt[:, b : b + 1],
                        in1=x[:, c0:m],
                        op0=mybir.AluOpType.is_gt,
                        op1=mybir.AluOpType.mult,
                        accum_out=acc[:, ci : ci + 1],
                    )
                    # ACT: S = sum relu(x - p) over ALL non-DVE cols [m, c1)
                    nc.scalar.activation(dummy_s[:], x[:, m : m + 1], copyf)
                    nc.scalar.activation(
                        scr[:, m:c1],
                        x[:, m:c1],
                        relu,
                        bias=pt[:, nblocks + b : nblocks + b + 1],
                        accum_out=acc[:, n_dve + ci : n_dve + ci + 1],
                    )
                    # ACT: G = sum sign(x - p) over [m, m2): count for the
                    # first uw non-DVE cols: cnt = (G + uw - ties)/2
                    if uw:
                        nc.scalar.activation(
                            scr[:, m:m2],
                            x[:, m:m2],
                            sign,
                            bias=pt[:, nblocks + b : nblocks + b + 1],
                            accum_out=acc[:, 2 * n_dve + ci : 2 * n_dve + ci + 1],
                        )
                    # Pool: cnt = sum [x > p] over the remaining [m2, c1)
                    nc.gpsimd.tensor_copy(dummy_g[:], x[:, m2 : m2 + 1])
                    nc.gpsimd.tensor_scalar(
                        out=scr_g[:, m2:c1],
                        in0=x[:, m2:c1],
                        scalar1=pt[:, b : b + 1],
                        scalar2=0.0,
                        op0=mybir.AluOpType.is_gt,
                        op1=mybir.AluOpType.add,
                        accum_out=acc[:, 3 * n_dve + ci : 3 * n_dve + ci + 1],
                    )
                elif split:
                    dw = dve_cols(c1 - c0)
                    m = c0 + dw
                    nc.vector.scalar_tensor_tensor(
                        out=scr[:, c0:m],
                        in0=x[:, c0:m],
                        scalar=pt[:, b : b + 1],
                        in1=x[:, c0:m],
                        op0=mybir.AluOpType.is_gt,
                        op1=mybir.AluOpType.mult,
                        accum_out=acc[:, ci : ci + 1],
                    )
                    nc.scalar.activation(
                        scr[:, m:c1],
                        x[:, m:c1],
                        relu,
                        bias=pt[:, nblocks + b : nblocks + b + 1],
                        accum_out=acc[:, n_dve + ci : n_dve + ci + 1],
                    )
                    nc.scalar.activation(
                        scr[:, m:c1],
                        x[:, m:c1],
                        sign,
                        bias=pt[:, nblocks + b : nblocks + b + 1],
                        accum_out=acc[:, 2 * n_dve + ci : 2 * n_dve + ci + 1],
                    )
                elif VARIANT == "ts2":
                    # For plain tensor_scalar with accum_out, op1 IS the
                    # reduce op: accum = reduce_op1(in0 op0 scalar1), then
                    # op1 scalar2.  Two add-reduce passes:
                    #   M = sum max(x, p);  N = sum [x > p]
                    # Host recovers A = sum x*[x>p] = M + p*(N - ncols).
                    nc.vector.tensor_scalar(
                        out=scr[:, c0:c1],
                        in0=x[:, c0:c1],
                        scalar1=pt[:, b : b + 1],
                        scalar2=0.0,
                        op0=mybir.AluOpType.max,
                        op1=mybir.AluOpType.add,
                        accum_out=acc[:, ci : ci + 1],
                    )
                    nc.vector.tensor_scalar(
                        out=scr[:, c0:c1],
                        in0=x[:, c0:c1],
                        scalar1=pt[:, b : b + 1],
                        scalar2=0.0,
                        op0=mybir.AluOpType.is_gt,
                        op1=mybir.AluOpType.add,
                        accum_out=accn[:, ci : ci + 1],
                    )
                else:
                    nc.vector.scalar_tensor_tensor(
                        out=scr[:, c0:c1],
                        in0=x[:, c0:c1],
                        scalar=pt[:, b : b + 1],
                        in1=x[:, c0:c1],
                        op0=mybir.AluOpType.is_gt,
                        op1=mybir.AluOpType.mult,
                        accum_out=acc[:, ci : ci + 1],
                    )
            nc.sync.dma_start(a_out[:], acc[:])
            if VARIANT == "ts2":
                nc.sync.dma_start(n_out[:], accn[:])
    # Legalize for TRN2 (at most 1 sem wait per instruction -> event sems).
    nc.compile()
    return nc


def _get_nc():
    key = (ROWS_PER_CORE, B)
    if key not in _NC_CACHE:
        _NC_CACHE[key] = build_bass()
    return _NC_CACHE[key]


def _act_col_ranges():
    """Per block: the (absolute-column) ranges counted via ACT Sign (whose
    exact ties need a host-side correction)."""
    nblocks = ROWS_PER_CORE // P
    chunks = chunk_plan(nblocks, B)
    out = {b: [] for b in range(nblocks)}
    for b, c0, c1 in chunks:
        m = c0 + dve_cols(c1 - c0)
        m2 = m + act_cnt_cols(c1 - c0) if VARIANT == "split3" else c1
        out[b].append((m, m2))
    return out


def _device_A(probs_q, p_q, eq_cnt=None, **run_kwargs):
    """Run the SPMD kernel on 8 cores; return A [B] float64 and the raw
    BassKernelResults (for profiling).  eq_cnt [B]: per-row count of exact
    ties q(x) == p within the ACT column ranges (split variant only)."""
    nblocks = ROWS_PER_CORE // P
    split = VARIANT in ("split", "split3")
    in_maps = []
    for k in range(N_CORES):
        r0 = k * ROWS_PER_CORE
        shard = _pack_shard(probs_q[r0 : r0 + ROWS_PER_CORE], nblocks, B)
        # p_true laid out [partition, block]: ptt[q, b] = p_q[r0 + b*P + q]
        ptt = np.ascontiguousarray(
            p_q[r0 : r0 + ROWS_PER_CORE].reshape(nblocks, P).T
        )
        if split:
            ptt = np.ascontiguousarray(np.concatenate([ptt, -ptt], axis=1))
        in_maps.append({"probs": shard, "p_true_t": ptt})
    res = run_bass_kernel_spmd(
        _get_nc(), in_maps, core_ids=list(range(N_CORES)), **run_kwargs
    )
    chunks = chunk_plan(nblocks, B)
    n_dve = len(chunks)
    A = np.empty(B, np.float64)
    for k in range(N_CORES):
        a = res.results[k]["a_out"]  # [P, nacc]
        if VARIANT == "ts2":
            a = np.concatenate([a, res.results[k]["n_out"]], axis=1)
        a_shard = np.zeros((nblocks, P), np.float64)
        n_act = np.zeros((nblocks, P), np.float64)
        pool_cnt = np.zeros((nblocks, P), np.float64)
        for ci, (b, c0, c1) in enumerate(chunks):
            a_shard[b] += a[:, ci].astype(np.float64)
            if VARIANT == "split3":
                a_shard[b] += a[:, n_dve + ci].astype(np.float64)  # S
                n_act[b] += a[:, 2 * n_dve + ci].astype(np.float64) + (
                    act_cnt_cols(c1 - c0)
                )
                pool_cnt[b] += a[:, 3 * n_dve + ci].astype(np.float64)
            elif split:
                # + S (relu accum); G (sign accum) + n_cols feeds the
                # count term below
                a_shard[b] += a[:, n_dve + ci].astype(np.float64)
                n_act[b] += a[:, 2 * n_dve + ci].astype(np.float64) + (
                    (c1 - c0) - dve_cols(c1 - c0)
                )
            elif VARIANT == "ts2":
                n_act[b] += a[:, n_dve + ci].astype(np.float64) - (c1 - c0)
        p_shard = (
            p_q[k * ROWS_PER_CORE : (k + 1) * ROWS_PER_CORE]
            .astype(np.float64)
            .reshape(nblocks, P)
        )
        if split:
            # count(x > p) = (G + n_sign_cols - ties) / 2  (+ Pool's count)
            eq = (
                eq_cnt[k * ROWS_PER_CORE : (k + 1) * ROWS_PER_CORE].reshape(
                    nblocks, P
                )
                if eq_cnt is not None
                else 0.0
            )
            a_shard += p_shard * ((n_act - eq) / 2.0 + pool_cnt)
        elif VARIANT == "ts2":
            a_shard += p_shard * n_act
        A[k * ROWS_PER_CORE : (k + 1) * ROWS_PER_CORE] = a_shard.reshape(-1)
    return A, res


def _same_label_correction(probs_u16, labels, p_q):
    """C[i] = sum over j with labels[j]==labels[i] of q*[q > qp[i]], exactly
    on the u16 integer scale (uint16 -> float64 is exact)."""
    C = np.zeros(B, np.float64)
    order = np.argsort(labels, kind="stable")
    ls = labels[order]
    bounds = np.flatnonzero(np.r_[True, ls[1:] != ls[:-1], True])
    for s, e in zip(bounds[:-1], bounds[1:]):
        g = order[s:e]
        sub = probs_u16[np.ix_(g, g)].astype(np.float64)
        pt = p_q[g].astype(np.float64)[:, None]
        C[g] = np.sum(np.where(sub > pt, sub, 0.0), axis=1)
    return C


def _exact_rows(probs, labels, p_true, rows):
    """Exact f32-input contrib for the given rows (float64 math)."""
    sub = probs[rows].astype(np.float64)
    pt = p_true[rows].astype(np.float64)[:, None]
    sel = (labels[None, :] != labels[rows][:, None]) & (sub > pt)
    den = np.where(sel, sub, 0.0).sum(axis=1)
    has = sel.any(axis=1)
    return np.where(has, p_true[rows].astype(np.float64) / (den + 1e-10), 0.0)


def run(probs, labels, **run_kwargs):
    """Full computation; returns (scalar ndarray float32, BassKernelResults)."""
    probs = np.ascontiguousarray(np.asarray(probs, dtype=np.float32))
    labels = np.asarray(labels).astype(np.int64)
    assert probs.shape == (B, B) and labels.shape == (B,)

    p_true = probs[np.arange(B), labels]  # f32 [B]
    # Quantize once; the SAME array feeds the device, the same-label
    # correction, and the suspect-row detection, so they agree exactly.
    if QMODE == "f16":
        probs_q = probs.astype(np.float16)
        p_q = p_true  # f32 scalar operand, compare is exact
        scale = np.float64(1.0)
    else:
        probs_q = np.rint(probs * QSCALE).astype(np.uint16)
        p_q = np.rint(p_true * QSCALE)  # integer-valued f32 scalar operand
        scale = np.float64(QSCALE)

    eq_cnt = None
    if VARIANT == "split":
        # sign(x - p) is 0 on exact ties, which only happen when p is
        # itself representable in the quantized dtype (~1 row in 8k);
        # count those ties exactly for the count reconstruction.
        eq_cnt = np.zeros(B, np.float64)
        rep = p_q == p_q.astype(probs_q.dtype).astype(p_q.dtype)
        if rep.any():
            ranges = _act_col_ranges()
            for i in np.flatnonzero(rep):
                b = (i % ROWS_PER_CORE) // P
                row = probs_q[i].astype(np.float64)
                eq_cnt[i] = sum(
                    float(np.sum(row[a0:a1] == np.float64(p_q[i])))
                    for a0, a1 in ranges[b]
                )

    A, res = _device_A(probs_q, p_q, eq_cnt, **run_kwargs)
    C = _same_label_correction(probs_q, labels, p_q)

    denom = (A - C) / scale
    contrib = np.where(
        denom > 0.25, p_true.astype(np.float64) / (denom + 1e-10), 0.0
    )
    suspect = denom < SUSPECT_T
    if suspect.any():
        rows = np.flatnonzero(suspect)
        contrib[rows] = _exact_rows(probs, labels, p_true, rows)
    out = np.float32(contrib.sum() / B)
    return np.array(out, dtype=np.float32), res


def kernel(probs, labels):
    out, _ = run(probs, labels)
    return out
